# revision 1
# baseline (speedup 1.0000x reference)
"""Trainium2 Bass kernel for nn_DPP: batched masked-Gram logdet minus shared
normalizer logdet.

out[i] = logdet(G * m_i m_i^T + diag(1-m_i)) - logdet(G + I),  G = B^T B

Sharding: data-parallel over the batch dim of x (one sample per NeuronCore,
B replicated). Each core computes its sample's masked-Gram logdet AND the
shared logdet(G+I) (redundantly -- no cross-core traffic); the host gathers
the 8 scalars.

Device algorithm (per core):
  - G = B^T B upper-triangle strips via bf16 matmuls (fp32 PSUM accum),
    emitted interleaved with the Cholesky panels so PE overlaps both.
  - Two interleaved left-looking blocked Cholesky factorizations (U-form,
    128-wide panels) of A1 = G*mm^T + diag(1-m) and A2 = G + I, never
    materialized: strips are formed from G on the fly.
  - Each 128x128 diagonal pivot S is handled matmul-only ("refine" scheme):
      d = diag(S); r = 1/sqrt(d)                  (DVE reciprocal + ACT Sqrt)
      corr = S * (r r^T); X1 = striu(corr); X1T = stril(corr)
      W = diag(r) (I - X1 + X1@X1)                (approx inv-chol factor)
      F = W^T S W - I                             (small: ||F|| ~ 0.15)
      logdet(S) = sum(ln d) + tr F - tr F^2/2 + tr F^3/3
      What = W + W(-F/2 + 3F^2/8)                 (What What^T ~ S^{-1} to O(F^3))
    Panel: U_strip = What^T @ strip; trailing Schur updates use U (bf16).
    All ln d are batched into one ACT Ln at the end (2 table loads total).
"""

import numpy as np
import ml_dtypes

P = 128
N = 2048           # padded matrix dim (= n columns of B)
NT = N // P        # 16 column tiles
NKT = 16           # contraction tiles (B rows padded 2000 -> 2048)
FT = 512           # free-dim tile for wide matmuls

_CACHE = {}


def _col_tiles(width_blocks, base_col, diag_first=False):
    """Split absolute cols [base_col, base_col + width_blocks*128) into <=512
    tiles. With diag_first, the first tile is exactly 128 wide (diag block)."""
    tiles = []
    c = base_col
    end = base_col + width_blocks * P
    if diag_first:
        tiles.append((c, P))
        c += P
    while c < end:
        w = min(FT, end - c)
        tiles.append((c, w))
        c += w
    return tiles


def _build():
    import concourse.bass as bass
    import concourse.bacc as bacc
    import concourse.mybir as mybir
    from concourse.bass import ds, ts
    from concourse.masks import (
        make_identity,
        make_upper_triangular,
        make_lower_triangular,
    )
    from concourse.tile import TileContext
    from contextlib import ExitStack

    f32 = mybir.dt.float32
    bf16 = mybir.dt.bfloat16
    AF = mybir.ActivationFunctionType
    OP = mybir.AluOpType
    PSUM = bass.MemorySpace.PSUM
    AX = mybir.AxisListType.X

    nc = bacc.Bacc()
    bb = nc.dram_tensor("bb", [N, N], bf16, kind="ExternalInput")
    mrow_d = nc.dram_tensor("mrow", [1, N], bf16, kind="ExternalInput")
    mcol_d = nc.dram_tensor("mcol", [N, 1], f32, kind="ExternalInput")
    out_d = nc.dram_tensor("out", [1, 1], f32, kind="ExternalOutput")

    with TileContext(nc) as tc, ExitStack() as stack:
        consts = stack.enter_context(tc.tile_pool(name="consts", bufs=1))
        I128 = consts.tile([P, P], f32, tag="i128")
        make_identity(nc, I128)
        I128b = consts.tile([P, P], bf16, tag="i128b")
        nc.vector.tensor_copy(I128b, I128)
        STRIU = consts.tile([P, P], f32, tag="striu")
        make_upper_triangular(nc, STRIU, val=1.0, diag=False)
        STRIL = consts.tile([P, P], f32, tag="stril")
        make_lower_triangular(nc, STRIL, val=1.0, diag=False)
        mrow = consts.tile([1, N], bf16, tag="mrow")
        nc.sync.dma_start(mrow, mrow_d[:, :])
        mcol = consts.tile([P, NT], f32, tag="mcol")
        nc.sync.dma_start(mcol, mcol_d.rearrange("(t p) one -> p (t one)", p=P))
        acc = consts.tile([P, 2], f32, tag="acc")
        nc.vector.memset(acc, 0.0)
        dstore = consts.tile([P, 2, NT], f32, tag="dstore")
        onem_all = consts.tile([P, NT], f32, tag="onem_all")
        nc.vector.tensor_scalar(
            out=onem_all, in0=mcol, scalar1=-1.0, scalar2=1.0,
            op0=OP.mult, op1=OP.add,
        )
        dfix_all = consts.tile([P, NT, P], f32, tag="dfix_all")
        for i in range(NT):
            nc.vector.tensor_scalar_mul(dfix_all[:, i, :], I128, onem_all[:, ds(i, 1)])

        gs = []  # gs[i]: [P, (NT-i)*P] bf16, absolute cols i*128..2048
        for i in range(NT):
            gs.append(consts.tile([P, (NT - i) * P], bf16, tag=f"gs{i}", name=f"gs{i}"))
        ub = {}  # ub[(m, i)]: [P, (NT-i)*P] bf16 panels of the two factorizations
        for m in range(2):
            for i in range(NT):
                ub[(m, i)] = consts.tile(
                    [P, (NT - i) * P], bf16, tag=f"ub{m}_{i}", name=f"ub{m}_{i}"
                )

        bpool = stack.enter_context(tc.tile_pool(name="bpool", bufs=1))
        gpsum = stack.enter_context(tc.tile_pool(name="gram_psum", bufs=2, space=PSUM))
        spool = stack.enter_context(tc.tile_pool(name="strip_pool", bufs=2))
        rpool = stack.enter_context(tc.tile_pool(name="ref_pool", bufs=2))
        vpool = stack.enter_context(tc.tile_pool(name="vec_pool", bufs=2))
        apsum = stack.enter_context(tc.tile_pool(name="acc_psum", bufs=2, space=PSUM))
        wpsum = stack.enter_context(tc.tile_pool(name="work_psum", bufs=4, space=PSUM))

        bt = bpool.tile([P, NKT, N], bf16, tag="bt")
        nc.sync.dma_start(bt, bb.rearrange("(t p) n -> p t n", p=P))

        def gram_chunks(i):
            """One yield per <=512-wide tile of Gram strip i (16-MM chain)."""
            for (c0, w) in _col_tiles(NT - i, i * P):
                pt = gpsum.tile([P, FT], f32, tag="gp", name="pt")
                for kt in range(NKT):
                    nc.tensor.matmul(
                        pt[:, :w],
                        bt[:, kt, ts(i, P)],
                        bt[:, kt, ds(c0, w)],
                        start=(kt == 0),
                        stop=(kt == NKT - 1),
                    )
                nc.scalar.copy(gs[i][:, ds(c0 - i * P, w)], pt[:, :w])
                yield

        def new_panel(i, m):
            wblk = NT - i
            return {
                "tiles": _col_tiles(wblk, i * P, diag_first=True),
                "strip": spool.tile([P, wblk * P], bf16, tag="strip", name="strip"),
                "sblk": rpool.tile([P, P], f32, tag="sblk", name="sblk"),
                "sb": rpool.tile([P, P], bf16, tag="sb", name="sb"),
            }

        def emit_accum_prep(i, m, cx, tix):
            """Accum psum chain + strip-prep for tile tix (diag tile: tix 0)."""
            c0, w = cx["tiles"][tix]
            is_diag = tix == 0
            strip, sblk, sb = cx["strip"], cx["sblk"], cx["sb"]
            ap = None
            if i > 0:
                ap = apsum.tile([P, FT], f32, tag="ap", name="ap")
                for j in range(i):
                    nc.tensor.matmul(
                        ap[:, :w],
                        ub[(m, j)][:, ds((i - j) * P, P)],
                        ub[(m, j)][:, ds(c0 - j * P, w)],
                        start=(j == 0),
                        stop=(j == i - 1),
                    )
            gsl = gs[i][:, ds(c0 - i * P, w)]
            if m == 0:
                mo = wpsum.tile([P, FT], f32, tag="w", name="mo")
                nc.tensor.matmul(
                    mo[:, :w], mrow[:, ts(i, P)], mrow[:, ds(c0, w)],
                    start=True, stop=True,
                )
                if is_diag:
                    tmp = rpool.tile([P, P], f32, tag="tmp", name="tmp")
                    nc.vector.tensor_mul(tmp, gsl, mo[:, :P])
                    if i > 0:
                        tmp2 = rpool.tile([P, P], f32, tag="tmp2", name="tmp2")
                        nc.vector.tensor_sub(tmp2, tmp, ap[:, :P])
                    else:
                        tmp2 = tmp
                    nc.vector.tensor_add(sblk, tmp2, dfix_all[:, i, :])
                    nc.vector.tensor_copy(sb, sblk)
                else:
                    tmp3 = spool.tile([P, FT], f32, tag="ptmp", name="tmp3")
                    nc.vector.tensor_mul(tmp3[:, :w], gsl, mo[:, :w])
                    if i > 0:
                        nc.vector.tensor_sub(
                            strip[:, ds(c0 - i * P, w)], tmp3[:, :w], ap[:, :w]
                        )
                    else:
                        nc.vector.tensor_copy(
                            strip[:, ds(c0 - i * P, w)], tmp3[:, :w]
                        )
            else:
                if is_diag:
                    if i > 0:
                        tmp = rpool.tile([P, P], f32, tag="tmp", name="tmp")
                        nc.vector.tensor_sub(tmp, gsl, ap[:, :P])
                        nc.vector.tensor_add(sblk, tmp, I128)
                    else:
                        nc.vector.tensor_add(sblk, gsl, I128)
                    nc.vector.tensor_copy(sb, sblk)
                else:
                    if i > 0:
                        nc.vector.tensor_sub(
                            strip[:, ds(c0 - i * P, w)], gsl, ap[:, :w]
                        )
                    # (m=1, i=0): TRSM reads gs[0] directly

        def refine_gen(i, m, cx):
            """Pivot-block factor; yields at cross-engine handoffs so filler
            matmuls can be emitted between dependent steps."""
            sblk, sb = cx["sblk"], cx["sb"]
            dcol = dstore[:, m, ds(i, 1)]
            dummy = rpool.tile([P, P], f32, tag="dummy", name="dummy")
            nc.vector.tensor_mul(dummy, sblk, I128)
            nc.vector.tensor_reduce(dcol, dummy, AX, OP.add)
            rinv = vpool.tile([P, 1], f32, tag="rinv", name="rinv")
            nc.vector.reciprocal(rinv, dcol)
            rcol = vpool.tile([P, 1], f32, tag="rcol", name="rcol")
            nc.scalar.sqrt(rcol, rinv)
            yield
            rt_ps = wpsum.tile([P, FT], f32, tag="w", name="rt_ps")
            nc.tensor.transpose(rt_ps[:1, :P], rcol, I128)
            rrow = vpool.tile([1, P], bf16, tag="rrow", name="rrow")
            nc.vector.tensor_copy(rrow, rt_ps[:1, :P])
            yield
            q_ps = wpsum.tile([P, FT], f32, tag="w", name="q_ps")
            nc.tensor.matmul(q_ps[:, :P], rrow, rrow, start=True, stop=True)
            c1 = rpool.tile([P, P], f32, tag="c1", name="c1")
            nc.vector.tensor_mul(c1, sblk, q_ps[:, :P])
            yield
            x1 = rpool.tile([P, P], bf16, tag="x1", name="x1")
            nc.gpsimd.tensor_mul(x1, c1, STRIU)
            x1t = rpool.tile([P, P], bf16, tag="x1t", name="x1t")
            nc.gpsimd.tensor_mul(x1t, c1, STRIL)
            yield
            x2_ps = wpsum.tile([P, FT], f32, tag="w", name="x2_ps")
            nc.tensor.matmul(x2_ps[:, :P], x1t, x1, start=True, stop=True)
            wser = rpool.tile([P, P], f32, tag="wser", name="wser")
            nc.vector.tensor_sub(wser, x2_ps[:, :P], x1)
            nc.vector.tensor_add(wser, wser, I128)
            wfac = rpool.tile([P, P], bf16, tag="wfac", name="wfac")
            nc.vector.tensor_scalar_mul(wfac, wser, rcol)
            yield
            wt_ps = wpsum.tile([P, FT * 2], bf16, tag="w", name="wt_ps")
            nc.tensor.transpose(wt_ps[:, :P], wfac, I128b)
            wt = rpool.tile([P, P], bf16, tag="wt", name="wt")
            nc.vector.tensor_copy(wt, wt_ps[:, :P])
            yield
            sw_ps = wpsum.tile([P, FT], f32, tag="w", name="sw_ps")
            nc.tensor.matmul(sw_ps[:, :P], sb, wfac, start=True, stop=True)
            swt = rpool.tile([P, P], bf16, tag="swt", name="swt")
            nc.vector.tensor_copy(swt, sw_ps[:, :P])
            yield
            fpi_ps = wpsum.tile([P, FT], f32, tag="w", name="fpi_ps")
            nc.tensor.matmul(fpi_ps[:, :P], wfac, swt, start=True, stop=True)
            ff = rpool.tile([P, P], bf16, tag="ff", name="ff")
            nc.vector.tensor_sub(ff, fpi_ps[:, :P], I128)
            trf = vpool.tile([P, 1], f32, tag="trf", name="trf")
            dummy3 = rpool.tile([P, P], f32, tag="dummy3", name="dummy3")
            nc.gpsimd.tensor_mul(dummy3, ff, I128)
            nc.vector.tensor_reduce(trf, dummy3, AX, OP.add)
            trf2 = vpool.tile([P, 1], f32, tag="trf2", name="trf2")
            dummy4 = rpool.tile([P, P], f32, tag="dummy4", name="dummy4")
            nc.gpsimd.tensor_mul(dummy4, ff, ff)
            nc.vector.tensor_reduce(trf2, dummy4, AX, OP.add)
            yield
            f2_ps = wpsum.tile([P, FT], f32, tag="w", name="f2_ps")
            nc.tensor.matmul(f2_ps[:, :P], ff, ff, start=True, stop=True)
            trf3 = vpool.tile([P, 1], f32, tag="trf3", name="trf3")
            dummy5 = rpool.tile([P, P], f32, tag="dummy5", name="dummy5")
            nc.vector.tensor_mul(dummy5, f2_ps[:, :P], ff)
            nc.vector.tensor_reduce(trf3, dummy5, AX, OP.add)
            f2s = rpool.tile([P, P], bf16, tag="f2s", name="f2s")
            nc.vector.tensor_scalar_mul(f2s, f2_ps[:, :P], 0.375)
            fs = rpool.tile([P, P], bf16, tag="fs", name="fs")
            nc.vector.tensor_scalar_mul(fs, ff, -0.5)
            yield
            wh_ps = wpsum.tile([P, FT], f32, tag="w", name="wh_ps")
            nc.tensor.matmul(wh_ps[:, :P], wt, fs, start=True, stop=False)
            nc.tensor.matmul(wh_ps[:, :P], wt, f2s, start=False, stop=True)
            what = rpool.tile([P, P], bf16, tag="what", name="what")
            nc.vector.tensor_add(what, wh_ps[:, :P], wfac)
            cx["what"] = what
            # logdet trace series accumulation
            t1 = vpool.tile([P, 1], f32, tag="t1", name="t1")
            t2 = vpool.tile([P, 1], f32, tag="t2", name="t2")
            nc.vector.tensor_scalar(
                out=t2, in0=trf2, scalar1=-0.5, scalar2=None, op0=OP.mult
            )
            nc.vector.tensor_add(t1, trf, t2)
            nc.vector.tensor_scalar(
                out=t2, in0=trf3, scalar1=1.0 / 3.0, scalar2=None, op0=OP.mult
            )
            nc.vector.tensor_add(t1, t1, t2)
            nc.vector.tensor_add(acc[:, ds(m, 1)], acc[:, ds(m, 1)], t1)

        def emit_trsm(i, m, cx):
            for tix, (c0, w) in enumerate(cx["tiles"]):
                if m == 1 and i == 0 and tix > 0:
                    rhs = gs[0][:, ds(c0, w)]
                elif tix == 0:
                    rhs = cx["sb"]
                else:
                    rhs = cx["strip"][:, ds(c0 - i * P, w)]
                tp = wpsum.tile([P, FT], f32, tag="w", name="tp")
                nc.tensor.matmul(tp[:, :w], cx["what"], rhs, start=True, stop=True)
                nc.scalar.copy(ub[(m, i)][:, ds(c0 - i * P, w)], tp[:, :w])

        # ---- interleaved emission: refine chains of both matrices zip, ----
        # ---- with Gram strips and trailing accumulations as PE filler  ----
        pending_fill = []
        pending_fill.extend(gram_chunks(0))  # strip 0 fully before panel 0
        for _ in gram_chunks(1):
            pass
        for i in range(NT):
            cxs = [new_panel(i, 0), new_panel(i, 1)]
            emit_accum_prep(i, 0, cxs[0], 0)
            emit_accum_prep(i, 1, cxs[1], 0)
            fillers = []
            if i + 2 < NT:
                fillers.append(gram_chunks(i + 2))
            def rest_chunks(m, cx):
                for tix in range(1, len(cx["tiles"])):
                    emit_accum_prep(i, m, cx, tix)
                    yield
            fillers.append(rest_chunks(0, cxs[0]))
            fillers.append(rest_chunks(1, cxs[1]))
            gens = [refine_gen(i, 0, cxs[0]), refine_gen(i, 1, cxs[1])]
            live = list(gens)
            fi = 0
            while live:
                for g in list(live):
                    try:
                        next(g)
                    except StopIteration:
                        live.remove(g)
                # one filler chunk between refine steps
                for _ in range(1):
                    while fillers:
                        try:
                            next(fillers[fi % len(fillers)])
                            break
                        except StopIteration:
                            fillers.pop(fi % len(fillers))
                    fi += 1
            # drain remaining fillers
            while fillers:
                g = fillers.pop(0)
                for _ in g:
                    pass
            emit_trsm(i, 0, cxs[0])
            emit_trsm(i, 1, cxs[1])

        # -------- final: batched Ln(d), partition-sum via matmul ------
        lnall = vpool.tile([P, 2, NT], f32, tag="lnall", name="lnall")
        nc.scalar.activation(
            lnall.rearrange("p a b -> p (a b)"),
            dstore.rearrange("p a b -> p (a b)"), AF.Ln,
        )
        ln0 = vpool.tile([P, 1], f32, tag="ln0", name="ln0")
        nc.vector.tensor_reduce(ln0, lnall[:, 0, :], AX, OP.add)
        ln1 = vpool.tile([P, 1], f32, tag="ln1", name="ln1")
        nc.vector.tensor_reduce(ln1, lnall[:, 1, :], AX, OP.add)
        accd = vpool.tile([P, 1], f32, tag="accd", name="accd")
        nc.vector.tensor_sub(accd, acc[:, 0:1], acc[:, 1:2])
        nc.vector.tensor_add(accd, accd, ln0)
        nc.vector.tensor_sub(accd, accd, ln1)
        ones = vpool.tile([P, 1], f32, tag="ones", name="ones")
        nc.vector.memset(ones, 1.0)
        r_ps = wpsum.tile([P, FT], f32, tag="w", name="r_ps")
        nc.tensor.matmul(r_ps[:1, :1], accd, ones, start=True, stop=True)
        res = vpool.tile([1, 1], f32, tag="res", name="res")
        nc.vector.tensor_copy(res, r_ps[:1, :1])
        nc.sync.dma_start(out_d[:, :], res)

    nc.finalize()
    return nc


def kernel(x, B):
    """Full inputs -> full output. x: [8, 2048] int32, B: [2000, 2048] f32."""
    from concourse.bass_utils import run_bass_kernel_spmd

    bs, n = x.shape
    k = B.shape[0]
    assert n == N and bs == 8

    if "nc" not in _CACHE:
        _CACHE["nc"] = _build()
    nc = _CACHE["nc"]

    bpad = np.zeros((N, N), dtype=ml_dtypes.bfloat16)
    bpad[:k, :] = B.astype(ml_dtypes.bfloat16)
    in_maps = []
    for c in range(bs):
        m = (x[c] == 1).astype(np.float32)
        in_maps.append({
            "bb": bpad,
            "mrow": m.astype(ml_dtypes.bfloat16).reshape(1, N),
            "mcol": m.reshape(N, 1).astype(np.float32),
        })
    res = run_bass_kernel_spmd(nc, in_maps, core_ids=list(range(bs)))
    out = np.array([r["out"][0, 0] for r in res.results], dtype=np.float32)
    return out



# revision 3
# speedup vs baseline: 1.0538x; 1.0538x over previous
"""Trainium2 Bass kernel for nn_DPP: batched masked-Gram logdet minus shared
normalizer logdet.

out[i] = logdet(G_sel_i) - logdet(G + I),  G = B^T B  (unit-norm columns)

Sharding (8 cores, one sample each):
  - Shared Gram G is SHARDED: core c computes square strips c and c+8 of G
    from fp8 B (DoubleRow matmuls), then two AllGathers (strips 0-7, 8-15)
    broadcast all strips; the collectives overlap with the masked-side work.
  - Masked term is COMPACT: the host gathers each sample's selected columns
    into Bsel [N, SB*128]; the device computes the compact Gram Bsel^T Bsel
    (equal to the selected submatrix of G) directly -- no mask vector ops,
    and the masked Cholesky shrinks from 16 panels to SB (~9).
  - logdet(G+I) is computed redundantly on every core (its Cholesky zips
    with the masked one to hide pivot-refinement latency).

Both factorizations use the baseline's matmul-only "refine" pivot scheme:
blocked left-looking U^T U Cholesky, 128-wide panels; each pivot block is
inverted approximately via a Neumann-type series with the logdet corrected
by tr F - tr F^2/2 + tr F^3/3.
"""

import numpy as np
import ml_dtypes

P = 128
N = 2048           # n (columns of B); also padded contraction dim (2000->2048)
NT = N // P        # 16 shared panels
NKT = 16           # contraction k-tiles
FT = 512           # free-dim tile for wide matmuls

_CACHE = {}


def _col_tiles(width_blocks, base_col, diag_first=False):
    tiles = []
    c = base_col
    end = base_col + width_blocks * P
    if diag_first:
        tiles.append((c, P))
        c += P
    while c < end:
        w = min(FT, end - c)
        tiles.append((c, w))
        c += w
    return tiles


def _build(SB):
    import concourse.bass as bass
    import concourse.bacc as bacc
    import concourse.mybir as mybir
    from concourse.bass import ds, ts
    from concourse.masks import (
        make_identity,
        make_upper_triangular,
        make_lower_triangular,
    )
    from concourse.tile import TileContext
    from contextlib import ExitStack
    from collections import deque

    f32 = mybir.dt.float32
    bf16 = mybir.dt.bfloat16
    f8 = mybir.dt.float8e4
    AF = mybir.ActivationFunctionType
    OP = mybir.AluOpType
    PSUM = bass.MemorySpace.PSUM
    AX = mybir.AxisListType.X
    DR = mybir.MatmulPerfMode.DoubleRow

    SP = SB * P
    NB = [SB, NT]          # panels per matrix: 0 = masked(compact), 1 = shared

    nc = bacc.Bacc()
    bb = nc.dram_tensor("bb", [N, N], f8, kind="ExternalInput")
    lhsg_d = nc.dram_tensor("lhsg", [N, 2 * P], f8, kind="ExternalInput")
    bsel_d = nc.dram_tensor("bsel", [N, SP], f8, kind="ExternalInput")
    vfix_d = nc.dram_tensor("vfix", [SP, 1], f32, kind="ExternalInput")
    out_d = nc.dram_tensor("out", [1, 1], f32, kind="ExternalOutput")

    with TileContext(nc) as tc, ExitStack() as stack:
        consts = stack.enter_context(tc.tile_pool(name="consts", bufs=1))
        I128 = consts.tile([P, P], f32, tag="i128")
        make_identity(nc, I128)
        I128b = consts.tile([P, P], bf16, tag="i128b")
        nc.vector.tensor_copy(I128b, I128)
        STRIU = consts.tile([P, P], f32, tag="striu")
        make_upper_triangular(nc, STRIU, val=1.0, diag=False)
        STRIL = consts.tile([P, P], f32, tag="stril")
        make_lower_triangular(nc, STRIL, val=1.0, diag=False)
        vcol = consts.tile([P, SB], f32, tag="vcol")
        nc.sync.dma_start(vcol, vfix_d.rearrange("(t p) one -> p (t one)", p=P))
        acc = consts.tile([P, 2], f32, tag="acc")
        nc.vector.memset(acc, 0.0)
        dstore = consts.tile([P, 2, NT], f32, tag="dstore")
        nc.vector.memset(dstore.rearrange("p a b -> p (a b)"), 1.0)
        dfixm = consts.tile([P, SB, P], f32, tag="dfixm")
        for i in range(SB):
            nc.vector.tensor_scalar_mul(dfixm[:, i, :], I128, vcol[:, ds(i, 1)])

        # masked gram strips; masked TRSM overwrites them in place (ub0 == mgs)
        mgs = []
        for i in range(SB):
            mgs.append(consts.tile([P, (SB - i) * P], bf16, tag=f"mgs{i}",
                                   name=f"mgs{i}"))
        ub1 = []
        for i in range(NT):
            ub1.append(consts.tile([P, (NT - i) * P], bf16, tag=f"ub1_{i}",
                                   name=f"ub1_{i}"))
        UB = [mgs, ub1]

        bpool = stack.enter_context(tc.tile_pool(name="bpool", bufs=1))
        gpsum = stack.enter_context(tc.tile_pool(name="gram_psum", bufs=2, space=PSUM))
        spool = stack.enter_context(tc.tile_pool(name="strip_pool", bufs=2))
        rpool = stack.enter_context(tc.tile_pool(name="ref_pool", bufs=2))
        vpool = stack.enter_context(tc.tile_pool(name="vec_pool", bufs=2))
        sstp = stack.enter_context(tc.tile_pool(name="sst_pool", bufs=2))
        apsum = stack.enter_context(tc.tile_pool(name="acc_psum", bufs=2, space=PSUM))
        wpsum = stack.enter_context(tc.tile_pool(name="work_psum", bufs=4, space=PSUM))
        dram = stack.enter_context(tc.tile_pool(name="dram", bufs=1, space="DRAM"))

        lhs = bpool.tile([P, NKT, 2 * P], f8, tag="lhs")
        nc.sync.dma_start(lhs, lhsg_d.rearrange("(t p) w -> p t w", p=P))
        bt = bpool.tile([P, NKT, N], f8, tag="bt")
        nc.sync.dma_start(bt, bb.rearrange("(t p) n -> p t n", p=P))
        bs = bpool.tile([P, NKT, SP], f8, tag="bs")
        nc.sync.dma_start(bs, bsel_d.rearrange("(t p) s -> p t s", p=P))

        cin = [dram.tile([P, N], bf16, tag=f"cin{h}", name=f"cin{h}") for h in range(2)]
        cout = [dram.tile([8, P, N], bf16, tag=f"cout{h}", name=f"cout{h}")
                for h in range(2)]

        # ---- phase A: this core's two square G strips + AllGathers ----
        for h in range(2):
            stt = spool.tile([P, N], bf16, tag="stt", name=f"stt{h}")
            for ft in range(N // FT):
                pt = gpsum.tile([P, FT], f32, tag="gp", name="pt")
                for kt in range(0, NKT, 2):
                    nc.tensor.matmul(
                        pt,
                        lhs[:, kt:kt + 2, ds(h * P, P)],
                        bt[:, kt:kt + 2, ds(ft * FT, FT)],
                        start=(kt == 0),
                        stop=(kt == NKT - 2),
                        perf_mode=DR,
                    )
                nc.scalar.copy(stt[:, ds(ft * FT, FT)], pt)
            nc.gpsimd.dma_start(cin[h], stt)
            nc.gpsimd.collective_compute(
                "AllGather",
                mybir.AluOpType.bypass,
                replica_groups=[list(range(8))],
                ins=[cin[h].opt()],
                outs=[cout[h].opt()],
            )

        # ---- masked compact gram strip generators (emitted as filler) ----
        def mgram_strip(i):
            for (c0, w) in _col_tiles(SB - i, i * P):
                pt = gpsum.tile([P, FT], f32, tag="gp", name="mgp")
                for kt in range(0, NKT, 2):
                    nc.tensor.matmul(
                        pt[:, :w],
                        bs[:, kt:kt + 2, ts(i, P)],
                        bs[:, kt:kt + 2, ds(c0, w)],
                        start=(kt == 0),
                        stop=(kt == NKT - 2),
                        perf_mode=DR,
                    )
                nc.scalar.copy(mgs[i][:, ds(c0 - i * P, w)], pt[:, :w])
                yield

        mg_gens = [mgram_strip(i) for i in range(SB)]
        fillers = deque(mg_gens[:])
        subcnt = [0]

        def force_gen(g):
            for _ in g:
                pass
            if g in fillers:
                fillers.remove(g)

        def filler_step():
            while fillers:
                try:
                    next(fillers[0])
                    return
                except StopIteration:
                    fillers.popleft()

        # ---- panel machinery (shared by both matrices) ----
        def emit_accum_prep(cx, tix):
            m, i = cx["m"], cx["i"]
            c0, w = cx["tiles"][tix]
            is_diag = tix == 0
            ap = None
            if i > 0:
                ap = apsum.tile([P, FT], f32, tag="ap", name="ap")
                for j in range(i):
                    nc.tensor.matmul(
                        ap[:, :w],
                        UB[m][j][:, ds((i - j) * P, P)],
                        UB[m][j][:, ds(c0 - j * P, w)],
                        start=(j == 0),
                        stop=(j == i - 1),
                    )
            gsl = cx["gsl"][:, ds(c0 - i * P, w)]
            if is_diag:
                dfix = dfixm[:, i, :] if m == 0 else I128
                if i > 0:
                    tmp = rpool.tile([P, P], f32, tag="tmp", name="tmp")
                    nc.vector.tensor_sub(tmp, gsl, ap[:, :P])
                    nc.vector.tensor_add(cx["sblk"], tmp, dfix)
                else:
                    nc.vector.tensor_add(cx["sblk"], gsl, dfix)
                nc.vector.tensor_copy(cx["sb"], cx["sblk"])
            else:
                if i > 0:
                    nc.vector.tensor_sub(
                        cx["strip"][:, ds(c0 - i * P, w)], gsl, ap[:, :w]
                    )
                # i == 0: TRSM reads gsl directly

        def start_panel(m, i):
            wblk = NB[m] - i
            cx = {"m": m, "i": i,
                  "tiles": _col_tiles(wblk, i * P, diag_first=True)}
            if m == 0:
                for j in range(i + 1):
                    force_gen(mg_gens[j])
                cx["gsl"] = mgs[i]
            else:
                sst = sstp.tile([P, N], bf16, tag="sst", name=f"sst{i}")
                nc.sync.dma_start(
                    sst[:, :wblk * P],
                    cout[i // 8][i % 8, :, ds(i * P, wblk * P)],
                )
                cx["gsl"] = sst
            cx["sblk"] = rpool.tile([P, P], f32, tag="sblk", name="sblk")
            cx["sb"] = rpool.tile([P, P], bf16, tag="sb", name="sb")
            cx["strip"] = spool.tile([P, wblk * P], bf16, tag="strip", name="strip")
            emit_accum_prep(cx, 0)

            def rest():
                for tix in range(1, len(cx["tiles"])):
                    emit_accum_prep(cx, tix)
                    yield
            cx["rest"] = rest()
            fillers.append(cx["rest"])
            return cx

        def refine_gen(m, i, cx):
            sblk, sb = cx["sblk"], cx["sb"]
            dcol = dstore[:, m, ds(i, 1)]
            dummy = rpool.tile([P, P], f32, tag="dummy", name="dummy")
            nc.vector.tensor_mul(dummy, sblk, I128)
            nc.vector.tensor_reduce(dcol, dummy, AX, OP.add)
            rinv = vpool.tile([P, 1], f32, tag="rinv", name="rinv")
            nc.vector.reciprocal(rinv, dcol)
            rcol = vpool.tile([P, 1], f32, tag="rcol", name="rcol")
            nc.scalar.sqrt(rcol, rinv)
            yield
            rt_ps = wpsum.tile([P, FT], f32, tag="w", name="rt_ps")
            nc.tensor.transpose(rt_ps[:1, :P], rcol, I128)
            rrow = vpool.tile([1, P], bf16, tag="rrow", name="rrow")
            nc.vector.tensor_copy(rrow, rt_ps[:1, :P])
            yield
            q_ps = wpsum.tile([P, FT], f32, tag="w", name="q_ps")
            nc.tensor.matmul(q_ps[:, :P], rrow, rrow, start=True, stop=True)
            c1 = rpool.tile([P, P], f32, tag="c1", name="c1")
            nc.vector.tensor_mul(c1, sblk, q_ps[:, :P])
            yield
            x1 = rpool.tile([P, P], bf16, tag="x1", name="x1")
            nc.gpsimd.tensor_mul(x1, c1, STRIU)
            x1t = rpool.tile([P, P], bf16, tag="x1t", name="x1t")
            nc.gpsimd.tensor_mul(x1t, c1, STRIL)
            yield
            x2_ps = wpsum.tile([P, FT], f32, tag="w", name="x2_ps")
            nc.tensor.matmul(x2_ps[:, :P], x1t, x1, start=True, stop=True)
            wser = rpool.tile([P, P], f32, tag="wser", name="wser")
            nc.vector.tensor_sub(wser, x2_ps[:, :P], x1)
            nc.vector.tensor_add(wser, wser, I128)
            wfac = rpool.tile([P, P], bf16, tag="wfac", name="wfac")
            nc.vector.tensor_scalar_mul(wfac, wser, rcol)
            yield
            wt_ps = wpsum.tile([P, FT * 2], bf16, tag="w", name="wt_ps")
            nc.tensor.transpose(wt_ps[:, :P], wfac, I128b)
            wt = rpool.tile([P, P], bf16, tag="wt", name="wt")
            nc.vector.tensor_copy(wt, wt_ps[:, :P])
            yield
            sw_ps = wpsum.tile([P, FT], f32, tag="w", name="sw_ps")
            nc.tensor.matmul(sw_ps[:, :P], sb, wfac, start=True, stop=True)
            swt = rpool.tile([P, P], bf16, tag="swt", name="swt")
            nc.vector.tensor_copy(swt, sw_ps[:, :P])
            yield
            fpi_ps = wpsum.tile([P, FT], f32, tag="w", name="fpi_ps")
            nc.tensor.matmul(fpi_ps[:, :P], wfac, swt, start=True, stop=True)
            ff = rpool.tile([P, P], bf16, tag="ff", name="ff")
            nc.vector.tensor_sub(ff, fpi_ps[:, :P], I128)
            trf = vpool.tile([P, 1], f32, tag="trf", name="trf")
            dummy3 = rpool.tile([P, P], f32, tag="dummy3", name="dummy3")
            nc.gpsimd.tensor_mul(dummy3, ff, I128)
            nc.vector.tensor_reduce(trf, dummy3, AX, OP.add)
            trf2 = vpool.tile([P, 1], f32, tag="trf2", name="trf2")
            dummy4 = rpool.tile([P, P], f32, tag="dummy4", name="dummy4")
            nc.gpsimd.tensor_mul(dummy4, ff, ff)
            nc.vector.tensor_reduce(trf2, dummy4, AX, OP.add)
            yield
            f2_ps = wpsum.tile([P, FT], f32, tag="w", name="f2_ps")
            nc.tensor.matmul(f2_ps[:, :P], ff, ff, start=True, stop=True)
            trf3 = vpool.tile([P, 1], f32, tag="trf3", name="trf3")
            dummy5 = rpool.tile([P, P], f32, tag="dummy5", name="dummy5")
            nc.vector.tensor_mul(dummy5, f2_ps[:, :P], ff)
            nc.vector.tensor_reduce(trf3, dummy5, AX, OP.add)
            f2s = rpool.tile([P, P], bf16, tag="f2s", name="f2s")
            nc.vector.tensor_scalar_mul(f2s, f2_ps[:, :P], 0.375)
            fs = rpool.tile([P, P], bf16, tag="fs", name="fs")
            nc.vector.tensor_scalar_mul(fs, ff, -0.5)
            yield
            wh_ps = wpsum.tile([P, FT], f32, tag="w", name="wh_ps")
            nc.tensor.matmul(wh_ps[:, :P], wt, fs, start=True, stop=False)
            nc.tensor.matmul(wh_ps[:, :P], wt, f2s, start=False, stop=True)
            what = rpool.tile([P, P], bf16, tag="what", name="what")
            nc.vector.tensor_add(what, wh_ps[:, :P], wfac)
            cx["what"] = what
            t1 = vpool.tile([P, 1], f32, tag="t1", name="t1")
            t2 = vpool.tile([P, 1], f32, tag="t2", name="t2")
            nc.vector.tensor_scalar(
                out=t2, in0=trf2, scalar1=-0.5, scalar2=None, op0=OP.mult
            )
            nc.vector.tensor_add(t1, trf, t2)
            nc.vector.tensor_scalar(
                out=t2, in0=trf3, scalar1=1.0 / 3.0, scalar2=None, op0=OP.mult
            )
            nc.vector.tensor_add(t1, t1, t2)
            nc.vector.tensor_add(acc[:, ds(m, 1)], acc[:, ds(m, 1)], t1)

        def emit_trsm(m, i, cx):
            force_gen(cx["rest"])
            for tix, (c0, w) in enumerate(cx["tiles"]):
                if tix == 0:
                    rhs = cx["sb"]
                elif i == 0:
                    rhs = cx["gsl"][:, ds(c0, w)]
                else:
                    rhs = cx["strip"][:, ds(c0 - i * P, w)]
                tp = wpsum.tile([P, FT], f32, tag="w", name="tp")
                nc.tensor.matmul(tp[:, :w], cx["what"], rhs, start=True, stop=True)
                nc.scalar.copy(UB[m][i][:, ds(c0 - i * P, w)], tp[:, :w])

        # ---- phase C: interleaved panel schedule ----
        GATE = 5          # shared panels start after this many masked panels
        mq = deque(range(SB))
        sq = deque(range(NT))
        live = {}
        started_m = 0
        while mq or sq or live:
            if 0 not in live and mq:
                i = mq.popleft()
                cx = start_panel(0, i)
                live[0] = (refine_gen(0, i, cx), cx)
                started_m += 1
            if 1 not in live and sq and (started_m >= GATE or not mq):
                i = sq.popleft()
                cx = start_panel(1, i)
                live[1] = (refine_gen(1, i, cx), cx)
            for m in (0, 1):
                if m in live:
                    g, cx = live[m]
                    try:
                        next(g)
                    except StopIteration:
                        emit_trsm(m, cx["i"], cx)
                        del live[m]
            filler_step()
        while fillers:
            force_gen(fillers[0])

        # ---- final: batched Ln(d), partition-sum via matmul ----
        lnall = vpool.tile([P, 2, NT], f32, tag="lnall", name="lnall")
        nc.scalar.activation(
            lnall.rearrange("p a b -> p (a b)"),
            dstore.rearrange("p a b -> p (a b)"), AF.Ln,
        )
        ln0 = vpool.tile([P, 1], f32, tag="ln0", name="ln0")
        nc.vector.tensor_reduce(ln0, lnall[:, 0, :], AX, OP.add)
        ln1 = vpool.tile([P, 1], f32, tag="ln1", name="ln1")
        nc.vector.tensor_reduce(ln1, lnall[:, 1, :], AX, OP.add)
        accd = vpool.tile([P, 1], f32, tag="accd", name="accd")
        nc.vector.tensor_sub(accd, acc[:, 0:1], acc[:, 1:2])
        nc.vector.tensor_add(accd, accd, ln0)
        nc.vector.tensor_sub(accd, accd, ln1)
        ones = vpool.tile([P, 1], f32, tag="ones", name="ones")
        nc.vector.memset(ones, 1.0)
        r_ps = wpsum.tile([P, FT], f32, tag="w", name="r_ps")
        nc.tensor.matmul(r_ps[:1, :1], accd, ones, start=True, stop=True)
        res = vpool.tile([1, 1], f32, tag="res", name="res")
        nc.vector.tensor_copy(res, r_ps[:1, :1])
        nc.sync.dma_start(out_d[:, :], res)

    nc.finalize()
    return nc


def prep_in_maps(x, B, SB):
    """Host-side sharding: per-core fp8 inputs."""
    f8 = ml_dtypes.float8_e4m3
    k, n = B.shape
    SPp = SB * P
    bpad8 = np.zeros((N, N), dtype=f8)
    bpad8[:k, :] = B.astype(f8)
    in_maps = []
    for c in range(8):
        sel = np.flatnonzero(x[c] == 1)
        s = len(sel)
        bsel = np.zeros((N, SPp), dtype=f8)
        bsel[:k, :s] = B[:, sel].astype(f8)
        vfix = np.zeros((SPp, 1), dtype=np.float32)
        vfix[s:] = 1.0
        lhsg = np.concatenate(
            [bpad8[:, c * P:(c + 1) * P], bpad8[:, (c + 8) * P:(c + 9) * P]],
            axis=1,
        )
        in_maps.append({
            "bb": bpad8, "lhsg": np.ascontiguousarray(lhsg),
            "bsel": bsel, "vfix": vfix,
        })
    return in_maps


def kernel(x, B):
    """Full inputs -> full output. x: [8, 2048] int32, B: [2000, 2048] f32."""
    from concourse.bass_utils import run_bass_kernel_spmd

    bs_, n = x.shape
    assert n == N and bs_ == 8
    s = (np.asarray(x) == 1).sum(axis=1)
    SB = max(2, -(-int(s.max()) // P))
    if SB not in _CACHE:
        _CACHE[SB] = _build(SB)
    nc = _CACHE[SB]
    in_maps = prep_in_maps(np.asarray(x), np.asarray(B, dtype=np.float32), SB)
    res = run_bass_kernel_spmd(nc, in_maps, core_ids=list(range(8)))
    return np.array([r["out"][0, 0] for r in res.results], dtype=np.float32)
